# revision 38
# baseline (speedup 1.0000x reference)
"""MoE (shared expert + 8 routed experts, top-2) on 8 Trainium2 NeuronCores.

Sharding: core c holds
  - shared-expert slice c: rows [c*1024, (c+1)*1024) of sw1/sw2 and the
    matching columns of sw3  -> partial (T, D) output, summed on host
  - routed expert c's weights (w12[c], w3[c]); host routes/gathers the
    tokens selected for expert c (capacity 1024 = the exact mean load),
    device computes unscaled expert outputs, host applies combine weights
    during the fp32 scatter-add; small per-expert overflows beyond the
    capacity are fixed up on host in fp32.

Precision: shared expert in bf16 (fp32 PSUM); routed experts in fp8 e4m3
with DoubleRow matmuls (K=256 per matmul, ~2x bf16 throughput, HW-measured
221ns per K=256xN=512 matmul). Scales: x*32, weights*512, h*8 -- all
power-of-2, max |value| < 240 so the e4m3/e4m3fn encoding ambiguity is
moot. The 1/(8*512) unscale is folded into the host-side combine weights.
Measured rel err 1.66e-2 (tolerance 2e-2); bf16-everywhere is 4.2e-3 and
fp8 anywhere in the shared expert exceeds 3.9e-2.

Scheduling: dummy warm-up matmuls hold the PE clock at 2.4 GHz through
the cold start and the phase boundary; DMA preloads are few large 3D
transfers emitted in first-use order (per-transfer ring overhead ~0.6us);
m=0,1 of the first block run k-major-interleaved so ~145 GB/s feeds the
PE from the first matmul; x is prefetched one chunk ahead of the output
writes on the ring. HW exec ~779us vs ~820us bf16 compute roofline with
~92% PE occupancy incl. fp8 speedup.
"""

import sys

if "/opt/trn_rl_repo" not in sys.path:
    sys.path.insert(0, "/opt/trn_rl_repo")

from contextlib import ExitStack

import numpy as np
import ml_dtypes

import concourse.bass as bass
import concourse.tile as tile
from concourse import mybir, bacc
from concourse.bass_utils import run_bass_kernel_spmd

BF16 = mybir.dt.bfloat16
F8 = mybir.dt.float8e4
F32 = mybir.dt.float32
AF = mybir.ActivationFunctionType
DR = mybir.MatmulPerfMode.DoubleRow
F8NP = ml_dtypes.float8_e4m3

# Problem shape (hardcoded per spec)
B, S, D = 2, 2048, 2048
T = B * S                  # 4096 tokens
E = 8                      # routed experts == n_cores
TOPK = 2
H_SHARED = 8192
HC = H_SHARED // 8         # shared-expert hidden slice per core
HR = 1024                  # routed expert hidden
NCORES = 8
NT = 512                   # token block (one PSUM bank at fp32)
P = 128

# fp8 scales (power-of-2; see module docstring)
SX = 32.0
SW = 512.0
SH = 8.0
# PSUM down-proj = (SH*h)*(SW*w3): host combine weights divided by SH*SW
COMB_DIV = SH * SW


def _build_program(C: int):
    """Build the SPMD Bass program for routed-token capacity C (multiple of 128)."""
    nc = bacc.Bacc("TRN2", target_bir_lowering=False, debug=False)

    xT = nc.dram_tensor("xT", [D, T], BF16, kind="ExternalInput")
    sw1T = nc.dram_tensor("sw1T", [D, HC], BF16, kind="ExternalInput")
    sw2T = nc.dram_tensor("sw2T", [D, HC], BF16, kind="ExternalInput")
    sw3T = nc.dram_tensor("sw3T", [HC, D], BF16, kind="ExternalInput")
    w12T = nc.dram_tensor("w12T", [D, 2 * HR], F8, kind="ExternalInput")
    w3T = nc.dram_tensor("w3T", [HR, D], F8, kind="ExternalInput")
    xgT = nc.dram_tensor("xgT", [D, C], F8, kind="ExternalInput")

    shared_outT = nc.dram_tensor("shared_outT", [D, T], F32, kind="ExternalOutput")
    # routed partials go back in bf16: the host combine is fp32 and the
    # 0.4% partial quantization adds <1e-4 to the rel metric in quadrature
    routed_outT = nc.dram_tensor("routed_outT", [D, C], BF16, kind="ExternalOutput")

    KD = D // P     # 16 contraction tiles over D
    KH = HC // P    # 8 contraction tiles over HC (== HR // P)
    KD2 = D // (2 * P)   # 8 fp8 DoubleRow contraction tiles over D
    KH2 = HR // (2 * P)  # 4 fp8 DoubleRow contraction tiles over HR

    with tile.TileContext(nc) as tc:
        with ExitStack() as ctx:
            # PE clock warm-up: the HAM un-throttles (1.2 -> 2.4 GHz) only
            # after ~3.4us of sustained PE activity. Run dummy matmuls on
            # scratch data while the first input DMAs stream, so the real
            # matmuls start warm. Scratch pools close before the real pools
            # so no SBUF/PSUM is held.
            with ExitStack() as ctx_w:
                wrm = ctx_w.enter_context(tc.tile_pool(name="wrm", bufs=1))
                psW = ctx_w.enter_context(tc.tile_pool(name="psW", bufs=1, space="PSUM"))
                scr = wrm.tile([P, NT], BF16, name="scr")
                nc.gpsimd.memset(scr[:], 0.0)
                pw = psW.tile([P, NT], F32, name="pw")
                for _ in range(8):
                    nc.tensor.matmul(pw[:], scr[:, 0:P], scr[:], start=True, stop=True)

            # pools that live across both phases (shapes shared via tags)
            wdn = ctx.enter_context(tc.tile_pool(name="w_dn", bufs=1))
            hpool = ctx.enter_context(tc.tile_pool(name="h_p", bufs=2))
            tpool = ctx.enter_context(tc.tile_pool(name="t_p", bufs=2))
            opool = ctx.enter_context(tc.tile_pool(name="o_p", bufs=4))
            psA = ctx.enter_context(tc.tile_pool(name="psA", bufs=2, space="PSUM"))
            psB = ctx.enter_context(tc.tile_pool(name="psB", bufs=4, space="PSUM"))

            sw3_all = wdn.tile([P, KH, D], BF16, name="sw3", tag="sw3")

            # 3D DRAM views: [p, k, col] with p the SBUF partition dim
            xT3 = xT[:, :].rearrange("(k p) t -> p k t", p=P)
            sw1T3 = sw1T[:, :].rearrange("(k p) h -> p k h", p=P)
            sw2T3 = sw2T[:, :].rearrange("(k p) h -> p k h", p=P)
            sw3T3 = sw3T[:, :].rearrange("(k p) d -> p k d", p=P)

            # ---------------- Phase A+B: shared expert ----------------
            # up-proj weights + x chunks live in an inner scope: closing it
            # before the routed phase lets w12/xg DMAs overlap the tail of
            # the shared down-proj (keeps PE fed and its clock warm).
            CH = 1024
            H2 = HC // 2
            with ExitStack() as ctx_ab:
                wup = ctx_ab.enter_context(tc.tile_pool(name="w_up", bufs=1))
                xpool = ctx_ab.enter_context(tc.tile_pool(name="x_ab", bufs=2))

                # weights split into column-half tiles: finer-grained WAR
                # release lets the routed-phase loads start a sub-block early.
                # DMAs are few large 3D transfers (per-transfer overhead is
                # ~0.6us on the HWDGE ring) emitted in first-use order: the
                # m-loop consumes 128-column weight slabs in sequence.
                sw1_sb = [wup.tile([P, KD, H2], BF16, name=f"sw1_{h}", tag=f"sw1_{h}")
                          for h in range(2)]
                sw2_sb = [wup.tile([P, KD, H2], BF16, name=f"sw2_{h}", tag=f"sw2_{h}")
                          for h in range(2)]
                x0_sb = xpool.tile([P, KD, CH], BF16, name="x", tag="x")
                # first transfers in exact consumption order and fine grain;
                # m=0 of sub-block 0 runs k-major-interleaved (2 matmuls per
                # x k-tile), so the stream only needs ~290 GB/s to keep the
                # PE fed from the very first matmul
                def slab(w_sb, wT3, m):
                    wh, wm = divmod(m, H2 // P)
                    nc.sync.dma_start(w_sb[wh][:, :, wm * P:(wm + 1) * P],
                                      wT3[:, :, m * P:(m + 1) * P])
                slab(sw1_sb, sw1T3, 0)
                nc.sync.dma_start(x0_sb[:, 0:2, 0:NT], xT3[:, 0:2, 0:NT])
                slab(sw2_sb, sw2T3, 0)
                nc.sync.dma_start(x0_sb[:, 2:4, 0:NT], xT3[:, 2:4, 0:NT])
                slab(sw1_sb, sw1T3, 1)
                slab(sw2_sb, sw2T3, 1)
                for kp in range(2, 8):
                    nc.sync.dma_start(x0_sb[:, 2 * kp:2 * kp + 2, 0:NT],
                                      xT3[:, 2 * kp:2 * kp + 2, 0:NT])
                for m in (2, 3):
                    slab(sw1_sb, sw1T3, m)
                    slab(sw2_sb, sw2T3, m)
                for kp in range(4):
                    nc.sync.dma_start(x0_sb[:, 4 * kp:4 * kp + 4, NT:CH],
                                      xT3[:, 4 * kp:4 * kp + 4, NT:CH])
                for m in (4, 5, 6, 7):
                    slab(sw1_sb, sw1T3, m)
                    slab(sw2_sb, sw2T3, m)
                for kp in range(2):
                    nc.sync.dma_start(sw3_all[:, kp * 4:(kp + 1) * 4, :],
                                      sw3T3[:, kp * 4:(kp + 1) * 4, :])

                # x loads are emitted one chunk ahead of the previous chunk's
                # output writes so the ring FIFO streams them early
                x_tiles = [x0_sb]
                for ch in range(T // CH):
                    if ch + 1 < T // CH:
                        xn = xpool.tile([P, KD, CH], BF16, name="x", tag="x")
                        for kp in range(2):
                            nc.sync.dma_start(
                                xn[:, kp * 8:(kp + 1) * 8, :],
                                xT3[:, kp * 8:(kp + 1) * 8,
                                    (ch + 1) * CH:(ch + 2) * CH])
                        x_tiles.append(xn)
                    x_sb = x_tiles[ch]
                    for sb in range(CH // NT):
                        otok = slice(ch * CH + sb * NT, ch * CH + (sb + 1) * NT)
                        stok = slice(sb * NT, (sb + 1) * NT)
                        hs = []
                        m_start = 0
                        if ch == 0 and sb == 0:
                            # k-major interleave over m=0: 2 matmuls per x
                            # k-tile keep the PE fed while the first-chunk
                            # DMAs are still streaming
                            m_start = 1
                            pg = psA.tile([P, NT], F32, name="pg", tag="pg")
                            pu = psA.tile([P, NT], F32, name="pu", tag="pu")
                            fill = psB.tile([P, NT], F32, name="po", tag="po")
                            for k in range(KD):
                                nc.tensor.matmul(pg[:], sw1_sb[0][:, k, 0:P],
                                                 x_sb[:, k, stok],
                                                 start=(k == 0), stop=(k == KD - 1))
                                nc.tensor.matmul(pu[:], sw2_sb[0][:, k, 0:P],
                                                 x_sb[:, k, stok],
                                                 start=(k == 0), stop=(k == KD - 1))
                                if k % 2 == 1 and k < 8:
                                    # fill matmuls on already-resident data:
                                    # the x stream can lag the k-loop here,
                                    # and an idle window would re-throttle
                                    # the PE clock during its warm-up
                                    for _ in range(2):
                                        nc.tensor.matmul(fill[:],
                                                         x_sb[:, 0, 0:P],
                                                         x_sb[:, 0, 0:NT],
                                                         start=True, stop=True)
                            sg = tpool.tile([P, NT], F32, name="sg", tag="sg")
                            nc.scalar.activation(sg[:], pg[:], AF.Silu)
                            h = hpool.tile([P, NT], BF16, name="h_0", tag="h_0")
                            nc.vector.tensor_mul(h[:], sg[:], pu[:])
                            hs.append(h)
                        for m in range(m_start, KH):
                            wh, wm = divmod(m, H2 // P)   # which weight half-tile
                            mm = slice(wm * P, (wm + 1) * P)
                            pg = psA.tile([P, NT], F32, name="pg", tag="pg")
                            pu = psA.tile([P, NT], F32, name="pu", tag="pu")
                            for k in range(KD):
                                nc.tensor.matmul(pg[:], sw1_sb[wh][:, k, mm],
                                                 x_sb[:, k, stok],
                                                 start=(k == 0), stop=(k == KD - 1))
                            for k in range(KD):
                                nc.tensor.matmul(pu[:], sw2_sb[wh][:, k, mm],
                                                 x_sb[:, k, stok],
                                                 start=(k == 0), stop=(k == KD - 1))
                            sg = tpool.tile([P, NT], F32, name="sg", tag="sg")
                            nc.scalar.activation(sg[:], pg[:], AF.Silu)
                            h = hpool.tile([P, NT], BF16, name=f"h_{m}", tag=f"h_{m}")
                            nc.vector.tensor_mul(h[:], sg[:], pu[:])
                            hs.append(h)
                        last_sb = (ch == T // CH - 1 and sb == CH // NT - 1)
                        for mo in range(KD):
                            po = psB.tile([P, NT], F32, name="po", tag="po")
                            for k in range(KH):
                                nc.tensor.matmul(po[:], sw3_all[:, k, mo * P:(mo + 1) * P],
                                                 hs[k][:],
                                                 start=(k == 0), stop=(k == KH - 1))
                            so = opool.tile([P, NT], F32, name="so", tag="so")
                            nc.vector.tensor_copy(so[:], po[:])
                            if last_sb:
                                # slack-rich writes go via SWDGE so the routed
                                # weight prefetch gets the HWDGE bandwidth
                                nc.gpsimd.dma_start(shared_outT[mo * P:(mo + 1) * P, otok], so[:])
                            else:
                                nc.sync.dma_start(shared_outT[mo * P:(mo + 1) * P, otok], so[:])

            # ---------------- Phase C+D: routed expert (fp8 DoubleRow) ----
            # allocated into the space freed by w_up/x_ab; DMAs overlap the
            # shared down-proj tail.  Tiles are [P, 2, F]: two 128-row
            # contraction planes per DoubleRow matmul (K=256).
            wcd = ctx.enter_context(tc.tile_pool(name="w_cd", bufs=1))
            xgp = ctx.enter_context(tc.tile_pool(name="xg_p", bufs=1))
            h8p = ctx.enter_context(tc.tile_pool(name="h8_p", bufs=2))

            w12_sb = wcd.tile([P, KD, 2 * HR], F8, name="w12", tag="w12")
            w3_sb = wcd.tile([P, KH, D], F8, name="w3", tag="w3")
            xg_sb = xgp.tile([P, KD, C], F8, name="xg", tag="xg")

            xgT3 = xgT[:, :].rearrange("(k p) t -> p k t", p=P)
            w12T3 = w12T[:, :].rearrange("(k p) m -> p k m", p=P)
            w3T3 = w3T[:, :].rearrange("(k p) m -> p k m", p=P)

            # emission in first-use order: xg k-parts feed the up-proj k-loop
            # immediately, then w12 column-slabs in m order, then w3
            for kp in range(2):
                nc.sync.dma_start(xg_sb[:, kp * 8:(kp + 1) * 8, :],
                                  xgT3[:, kp * 8:(kp + 1) * 8, :])
            for m in range(KH):
                nc.sync.dma_start(w12_sb[:, :, m * P:(m + 1) * P],
                                  w12T3[:, :, m * P:(m + 1) * P])
                nc.sync.dma_start(w12_sb[:, :, HR + m * P:HR + (m + 1) * P],
                                  w12T3[:, :, HR + m * P:HR + (m + 1) * P])
            for kp in range(2):
                nc.sync.dma_start(w3_sb[:, kp * 4:(kp + 1) * 4, :],
                                  w3T3[:, kp * 4:(kp + 1) * 4, :])

            # bridge the phase boundary with dummy matmuls: the routed
            # weights may land a few us after the shared tail drains, and a
            # >3.4us PE-idle window would re-throttle the clock to 1.2 GHz
            pw2 = psB.tile([P, NT], F32, name="po", tag="po")
            for _ in range(12):
                nc.tensor.matmul(pw2[:], sw3_all[:, 0, 0:P], sw3_all[:, 0, 0:NT],
                                 start=True, stop=True)

            # block widths: full 512s plus one narrow remainder block
            widths = [NT] * (C // NT)
            if C % NT:
                widths.append(C % NT)
            # All up-projections first, then all down-projections: when the
            # down matmuls run, every h8 plane is long since written, so the
            # PE never bubbles on the activation/mul chain of the last m.
            # (Needs one h8 buffer per block -- with more blocks than the 2
            # buffers the passes still interleave correctly through WAR
            # semaphores, just with less overlap; C <= 1024 gives 2 blocks.)

            def up_pass(nt, tok, h8):
                for m in range(KH):
                    c1 = slice(m * P, (m + 1) * P)              # gate rows
                    c2 = slice(HR + m * P, HR + (m + 1) * P)    # up rows
                    p1 = psA.tile([P, NT], F32, name="pg", tag="pg")
                    p2 = psA.tile([P, NT], F32, name="pu", tag="pu")
                    for k in range(KD2):
                        nc.tensor.matmul(p1[:, :nt], w12_sb[:, 2 * k:2 * k + 2, c1],
                                         xg_sb[:, 2 * k:2 * k + 2, tok],
                                         start=(k == 0), stop=(k == KD2 - 1),
                                         perf_mode=DR)
                    for k in range(KD2):
                        nc.tensor.matmul(p2[:, :nt], w12_sb[:, 2 * k:2 * k + 2, c2],
                                         xg_sb[:, 2 * k:2 * k + 2, tok],
                                         start=(k == 0), stop=(k == KD2 - 1),
                                         perf_mode=DR)
                    # p1 = 16384*g, p2 = 16384*u; h8 = 8*silu(g)*u in e4m3
                    sg = tpool.tile([P, NT], F32, name="sg", tag="sg")
                    nc.scalar.activation(sg[:, :nt], p1[:, :nt], AF.Silu,
                                         scale=2.0 ** -14)
                    up = tpool.tile([P, NT], F32, name="up", tag="up")
                    nc.scalar.activation(up[:, :nt], p2[:, :nt], AF.Copy,
                                         scale=2.0 ** -11)
                    nc.vector.tensor_mul(h8[:, m, :nt], sg[:, :nt], up[:, :nt])

            def down_pass(nt, tok, h8, last_blk):
                for mo in range(KD):
                    po = psB.tile([P, NT], F32, name="po", tag="po")
                    for k in range(KH2):
                        nc.tensor.matmul(po[:, :nt],
                                         w3_sb[:, 2 * k:2 * k + 2, mo * P:(mo + 1) * P],
                                         h8[:, 2 * k:2 * k + 2, :nt],
                                         start=(k == 0), stop=(k == KH2 - 1),
                                         perf_mode=DR)
                    so = opool.tile([P, NT], BF16, name="so", tag="so")
                    # tail of the kernel: alternate the PSUM->SBUF copies
                    # between vector and scalar so the final chain is short;
                    # all stores stay on the low-latency sync HWDGE ring
                    # (SWDGE stores cost a ~4us gpsimd drain at NEFF teardown)
                    if last_blk and mo % 2 == 1:
                        nc.scalar.activation(so[:, :nt], po[:, :nt], AF.Copy)
                    else:
                        nc.vector.tensor_copy(so[:, :nt], po[:, :nt])
                    nc.sync.dma_start(routed_outT[mo * P:(mo + 1) * P, tok],
                                      so[:, :nt])

            blocks = []
            off = 0
            for nt in widths:
                tok = slice(off, off + nt)
                off += nt
                h8 = h8p.tile([P, KH, NT], F8, name="h8", tag="h8")
                blocks.append((nt, tok, h8))
                up_pass(nt, tok, h8)
            for bi, (nt, tok, h8) in enumerate(blocks):
                down_pass(nt, tok, h8, bi == len(blocks) - 1)

    nc.compile()
    return nc


_PROGRAM_CACHE: dict = {}


def _get_program(C: int):
    if C not in _PROGRAM_CACHE:
        _PROGRAM_CACHE[C] = _build_program(C)
    return _PROGRAM_CACHE[C]


def _route_like_reference(xf: np.ndarray, router_w: np.ndarray,
                          expert_bias: np.ndarray):
    """Router computed with jax on CPU to bit-match the reference's top-k."""
    import jax
    import jax.numpy as jnp

    cpu = jax.devices("cpu")[0]
    with jax.default_device(cpu):
        xj = jnp.asarray(xf)
        scores = jax.nn.sigmoid(xj @ jnp.asarray(router_w).T)        # (T, E)
        sel = scores + jnp.asarray(expert_bias)
        _, top_idx = jax.lax.top_k(sel, TOPK)                        # (T, K)
        top_sc = jnp.take_along_axis(scores, top_idx, axis=-1)
        top_w = top_sc / (top_sc.sum(-1, keepdims=True) + 1e-9)
        return np.asarray(top_idx), np.asarray(top_w)


def kernel(x, w12, w3, router_w, expert_bias, sw1, sw2, sw3):
    x = np.asarray(x, dtype=np.float32)
    w12 = np.asarray(w12, dtype=np.float32)
    w3 = np.asarray(w3, dtype=np.float32)
    router_w = np.asarray(router_w, dtype=np.float32)
    expert_bias = np.asarray(expert_bias, dtype=np.float32)
    sw1 = np.asarray(sw1, dtype=np.float32)
    sw2 = np.asarray(sw2, dtype=np.float32)
    sw3 = np.asarray(sw3, dtype=np.float32)

    xf = x.reshape(T, D)
    top_idx, top_w = _route_like_reference(xf, router_w, expert_bias)

    # per-expert token lists + combine weights
    idx_list, w_list = [], []
    for e in range(E):
        hit = top_idx == e                      # (T, K)
        tok = np.nonzero(hit.any(axis=1))[0]
        wt = (top_w * hit).sum(axis=1)[tok]     # combine weight per token
        idx_list.append(tok.astype(np.int64))
        w_list.append(wt.astype(np.float32))

    max_n = max(len(i) for i in idx_list)
    # Device capacity policy: cap at C_CORE (the exact mean load for top-2 of
    # 8 experts) and fix up small per-expert overflows on host in fp32
    # (<0.2% of FLOPs, like the router). Grossly imbalanced routing falls
    # back to extra device launches in slabs of C_MAX.
    C_CORE = 1024
    C_MAX = 1280   # slab size for the imbalanced-routing fallback (SBUF limit)
    overflow = sum(max(0, len(i) - C_CORE) for i in idx_list)
    if max_n <= C_CORE:
        C = max(2 * P, -(-max_n // (2 * P)) * 2 * P)    # capacity, multiple of 256
        n_launches, host_fix = 1, False
    elif overflow <= 1024:
        C, n_launches, host_fix = C_CORE, 1, True
    else:
        C = C_MAX
        n_launches, host_fix = max(1, -(-max_n // C_MAX)), False

    xT16 = np.ascontiguousarray(xf.T).astype(ml_dtypes.bfloat16)   # (D, T)
    xT8 = (np.ascontiguousarray(xf.T) * SX).astype(F8NP)           # (D, T) fp8

    nc = _get_program(C)

    sw_z = np.zeros((D, HC), dtype=ml_dtypes.bfloat16)
    sw3_z = np.zeros((HC, D), dtype=ml_dtypes.bfloat16)

    outT = np.zeros((D, T), dtype=np.float32)
    global _LAST_RESULTS
    for launch in range(n_launches):
        lo = launch * C_MAX
        in_maps = []
        for c in range(NCORES):
            hs = slice(c * HC, (c + 1) * HC)
            idx_c = idx_list[c][lo:lo + C]
            n_c = len(idx_c)
            xg = np.zeros((D, C), dtype=F8NP)
            xg[:, :n_c] = xT8[:, idx_c]
            if launch == 0:
                s1 = np.ascontiguousarray(sw1[hs].T).astype(ml_dtypes.bfloat16)
                s2 = np.ascontiguousarray(sw2[hs].T).astype(ml_dtypes.bfloat16)
                s3 = np.ascontiguousarray(sw3[:, hs].T).astype(ml_dtypes.bfloat16)
            else:
                s1, s2, s3 = sw_z, sw_z, sw3_z   # shared part already done
            in_maps.append({
                "xT": xT16,
                "sw1T": s1, "sw2T": s2, "sw3T": s3,
                "w12T": np.ascontiguousarray(w12[c].T * SW).astype(F8NP),
                "w3T": np.ascontiguousarray(w3[c].T * SW).astype(F8NP),
                "xgT": xg,
            })

        res = run_bass_kernel_spmd(nc, in_maps, core_ids=list(range(NCORES)),
                                   **_RUN_KWARGS)
        _LAST_RESULTS = res

        for c in range(NCORES):
            if launch == 0:
                outT += res.results[c]["shared_outT"]
            idx_c = idx_list[c][lo:lo + C]
            if len(idx_c):
                # token indices are unique within one expert; combine weight
                # (pre-divided by the fp8 scale product) applied here in fp32
                wdev = w_list[c][lo:lo + C] / COMB_DIV
                rout = res.results[c]["routed_outT"][:, :len(idx_c)].astype(np.float32)
                outT[:, idx_c] += rout * wdev[None, :]

    if host_fix:
        # fp32 fixup for tokens beyond the device capacity of each expert
        for c in range(NCORES):
            tail = idx_list[c][C:]
            if len(tail) == 0:
                continue
            wts = w_list[c][C:]
            xs = xf[tail]                             # (n, D)
            h12 = xs @ w12[c].T                       # (n, 2*HR)
            h1, h2 = h12[:, :HR], h12[:, HR:]
            h = h1 / (1.0 + np.exp(-h1)) * h2         # silu(h1) * h2
            out = (h * wts[:, None]) @ w3[c].T        # (n, D)
            outT[:, tail] += out.T
    return outT.T.reshape(B, S, D).astype(np.float32)


# test harness hooks: set _RUN_KWARGS = {"trace": True, ...} before calling
# kernel() to profile; read _LAST_RESULTS afterwards.
_RUN_KWARGS: dict = {}
_LAST_RESULTS = None
